# revision 1
# baseline (speedup 1.0000x reference)
# Trainium2 Bass kernel: causal single-head attention
#   out = softmax(causal(x @ W_qk.T @ x.T)) @ x @ W_ov.T
# n_context=4096, d_model=2048, distributed over 8 NeuronCores.
#
# Sharding: sequence-parallel over query rows with causal load balancing.
# The 4096 queries are split into 32 chunks of 128 rows. Core i owns chunks
# {8*(s+1)-1-i : s=0..3}, one per "slot" s. Slot s processes a fixed key
# prefix of L[s] = 8*(s+1) key-blocks (128 keys each) on every core, so all
# cores run the identical instruction stream (SPMD) while the causal work is
# balanced: each core computes 8+16+24+32 = 80 key-blocks of scores.
# Keys beyond a chunk's causal limit are neutralized with an additive -1e30
# mask streamed from the host (per-core data); only the last two key groups
# of each slot can straddle the diagonal, earlier groups are always valid.
#
# Precision: q-projection and scores run on the TensorEngine in float32r
# (fp32 with 11-bit mantissa, full-rate); value path (attn @ x and the
# output projection) runs in bfloat16 with fp32 PSUM accumulation.
import os

import numpy as np
import ml_dtypes

import concourse.bass as bass
import concourse.tile as tile
from concourse import bacc, mybir
from concourse import masks as cmasks
from concourse.bass_utils import run_bass_kernel_spmd

F32 = mybir.dt.float32
FR = mybir.dt.float32r
BF = mybir.dt.bfloat16
AL = mybir.AluOpType
AF = mybir.ActivationFunctionType

N_CTX, D = 4096, 2048
P = 128
NCORES = 8
NSLOT = 4
L = [8, 16, 24, 32]            # key blocks per slot
GRP = [2, 4, 6, 8]             # 512-wide key groups per slot
DK = D // P                    # 16 contraction chunks of 128
NJB = 32                       # key blocks overall
VISITS = [(g, s) for g in range(8) for s in (3, 2, 1, 0) if g < GRP[s]]
# only the last two key groups of a slot can contain the causal boundary
VISITS_MASKED = [(g, s) for (g, s) in VISITS if g >= 2 * s]
MASK_NEG = -1.0e30

bfloat16 = ml_dtypes.bfloat16


def _chunk_of(core, s):
    return 8 * (s + 1) - 1 - core


def _round_fp32r(a):
    bits = np.ascontiguousarray(a, dtype=np.float32).view(np.uint32)
    rounded = (bits + np.uint32(0x7FF) + ((bits >> np.uint32(12)) & np.uint32(1))) & np.uint32(0xFFFFF000)
    return rounded.view(np.float32)


def _n_jb(jb):
    # moving width of the attn@x matmul for key block jb: active slots form a
    # contiguous column prefix (slot order 3,2,1,0 in attnT)
    if jb < 8:
        return 512
    if jb < 16:
        return 384
    if jb < 24:
        return 256
    return 128


def build_graph():
    nc = bacc.Bacc("TRN2", target_bir_lowering=False, debug=False, num_devices=NCORES)
    xq_e = nc.dram_tensor("xq", [D, 512], FR, kind="ExternalInput").ap()
    wqk_e = nc.dram_tensor("wqk", [D, D], FR, kind="ExternalInput").ap()
    xk_e = nc.dram_tensor("xk", [D, N_CTX], FR, kind="ExternalInput").ap()
    xv_e = nc.dram_tensor("xv", [DK, NJB // 8, P, 8, P], BF, kind="ExternalInput").ap()
    wov_e = nc.dram_tensor("wov", [D, D], BF, kind="ExternalInput").ap()
    mask_e = nc.dram_tensor(
        "mask", [len(VISITS_MASKED), P, 512], F32, kind="ExternalInput").ap()
    out_e = nc.dram_tensor("out", [D, 512], F32, kind="ExternalOutput").ap()

    with tile.TileContext(nc) as tc:
        with (
            tc.tile_pool(name="const", bufs=1) as const_pool,
            tc.tile_pool(name="attnT", bufs=NJB) as at_pool,
            tc.tile_pool(name="small", bufs=16) as small_pool,
            tc.tile_pool(name="xv", bufs=5) as xv_pool,
            tc.tile_pool(name="ps512", bufs=6, space="PSUM") as ps_pool,
            tc.tile_pool(name="tp", bufs=2, space="PSUM") as tp_pool,
        ):
            ident = const_pool.tile([P, P], BF, tag="ident")

            with tc.tile_pool(name="qt", bufs=DK) as qt_pool:
                # ------------- phase A: q projection (qT = W_qk @ xq.T) -------------
                qt = [None] * DK
                with (
                    tc.tile_pool(name="xq", bufs=DK) as xq_pool,
                    tc.tile_pool(name="wqk", bufs=DK) as wqk_pool,
                ):
                    xq_t = [None] * DK
                    for mh in range(2):
                        pairs = []
                        for kc in range(DK):
                            if mh == 0:
                                xq_t[kc] = xq_pool.tile([P, 512], FR, tag="xq", name="xq")
                                nc.sync.dma_start(
                                    xq_t[kc][:], xq_e[kc * P:(kc + 1) * P, :])
                            wq = wqk_pool.tile([P, 1024], FR, tag="wqk", name="wq")
                            nc.sync.dma_start(
                                wq[:],
                                wqk_e[kc * P:(kc + 1) * P, mh * 1024:(mh + 1) * 1024])
                            pairs.append(wq)
                        for half in range(2):
                            mq = mh * 2 + half
                            qp = [ps_pool.tile([P, 512], F32, tag="ps512", name="qp")
                                  for _ in range(4)]
                            for kc in range(DK):
                                for m4 in range(4):
                                    nc.tensor.matmul(
                                        qp[m4][:],
                                        lhsT=pairs[kc][:, half * 512 + m4 * P:
                                                       half * 512 + (m4 + 1) * P],
                                        rhs=xq_t[kc][:],
                                        start=(kc == 0), stop=(kc == DK - 1))
                            for m4 in range(4):
                                m = mq * 4 + m4
                                qt[m] = qt_pool.tile([P, 512], FR, tag="qt", name="qt")
                                nc.vector.tensor_copy(qt[m][:], qp[m4][:])

                # ------------- phase B: scores + softmax, key-group major -------------
                attnT = [at_pool.tile([P, 512], BF, tag="attnT", name="attnT")
                         for _ in range(NJB)]
                cmasks.make_identity(nc, ident[:])
                with (
                    tc.tile_pool(name="xk", bufs=22) as xk_pool,
                    tc.tile_pool(name="maskp", bufs=3) as mask_pool,
                    tc.tile_pool(name="stripes", bufs=1) as stripe_pool,
                    tc.tile_pool(name="attn", bufs=1) as attn_pool,
                ):
                    stripes = [
                        stripe_pool.tile([P, L[s] * P], F32, tag=f"str{s}",
                                         name=f"stripe{s}")
                        for s in range(NSLOT)
                    ]
                    attn_s = [None] * NSLOT
                    negmp = [None] * NSLOT   # partial row-max (negated), slots 2/3

                    def softmax_and_transpose(s):
                        negmax = small_pool.tile([P, 1], F32, tag="small", name="negmax")
                        if negmp[s] is None:
                            nc.vector.tensor_reduce(
                                negmax[:], stripes[s][:], axis=mybir.AxisListType.X,
                                op=AL.max, negate=True)
                        else:
                            # combine the early partial max with the last two groups
                            cols_p = (GRP[s] - 2) * 512
                            negmf = small_pool.tile([P, 1], F32, tag="small", name="negmf")
                            nc.vector.tensor_reduce(
                                negmf[:], stripes[s][:, cols_p:],
                                axis=mybir.AxisListType.X, op=AL.max, negate=True)
                            nc.vector.tensor_tensor(
                                out=negmax[:], in0=negmp[s][:], in1=negmf[:], op=AL.min)
                        attn_s[s] = attn_pool.tile(
                            [P, L[s] * P], BF, tag=f"attn{s}", name=f"attn{s}")
                        ssum = small_pool.tile([P, 1], F32, tag="small", name="ssum")
                        nc.scalar.activation(
                            attn_s[s][:], stripes[s][:], AF.Exp,
                            bias=negmax[:], scale=1.0, accum_out=ssum[:])
                        rs = small_pool.tile([P, 1], F32, tag="small", name="rs")
                        nc.vector.reciprocal(rs[:], ssum[:])
                        # normalize per query row (partition dim) before transposing
                        nc.vector.tensor_scalar_mul(attn_s[s][:], attn_s[s][:], rs[:])
                        for jb in range(L[s]):
                            tp = tp_pool.tile([P, P], BF, tag="tp", name="tp")
                            nc.tensor.transpose(
                                tp[:], attn_s[s][:, jb * P:(jb + 1) * P], ident[:])
                            nc.vector.tensor_copy(
                                attnT[jb][:, (3 - s) * P:(4 - s) * P], tp[:])

                    for g in range(8):
                        xk_t = []
                        for kc in range(DK):
                            t = xk_pool.tile([P, 512], FR, tag="xk", name="xk")
                            nc.sync.dma_start(
                                t[:], xk_e[kc * P:(kc + 1) * P, g * 512:(g + 1) * 512])
                            xk_t.append(t)
                        for s in (3, 2, 1, 0):
                            if g >= GRP[s]:
                                continue
                            sc = ps_pool.tile([P, 512], F32, tag="ps512", name="sc")
                            for kc in range(DK):
                                nc.tensor.matmul(
                                    sc[:],
                                    lhsT=qt[kc][:, s * P:(s + 1) * P],
                                    rhs=xk_t[kc][:],
                                    start=(kc == 0), stop=(kc == DK - 1))
                            dst = stripes[s][:, g * 512:(g + 1) * 512]
                            if (g, s) in VISITS_MASKED:
                                v = VISITS_MASKED.index((g, s))
                                mt = mask_pool.tile([P, 512], F32, tag="maskp", name="mt")
                                nc.sync.dma_start(mt[:], mask_e[v])
                                nc.vector.tensor_tensor(
                                    out=dst, in0=sc[:], in1=mt[:], op=AL.add)
                            else:
                                nc.vector.tensor_copy(dst, sc[:])
                        # early partial row-max for the big slots, off the critical path
                        for s in (2, 3):
                            if g == GRP[s] - 3:
                                negmp[s] = small_pool.tile(
                                    [P, 1], F32, tag="small", name="negmp")
                                nc.vector.tensor_reduce(
                                    negmp[s][:], stripes[s][:, 0:(GRP[s] - 2) * 512],
                                    axis=mybir.AxisListType.X, op=AL.max, negate=True)
                        for s in range(NSLOT):
                            if GRP[s] - 1 == g:
                                softmax_and_transpose(s)

            # ------------- phase C: attn @ x (yT) + output projection -------------
            with (
                tc.tile_pool(name="yt", bufs=DK) as yt_pool,
                tc.tile_pool(name="wov", bufs=DK) as wov_pool,
                tc.tile_pool(name="osb", bufs=3) as o_pool,
            ):
                yt = [None] * DK
                for dm in range(DK):
                    yp = ps_pool.tile([P, 512], F32, tag="ps512", name="yp")
                    for jb8 in range(NJB // 8):
                        xvt = xv_pool.tile([P, 8, P], BF, tag="xv", name="xvt")
                        nc.sync.dma_start(xvt[:], xv_e[dm, jb8])
                        for jl in range(8):
                            jb = jb8 * 8 + jl
                            njb = _n_jb(jb)
                            nc.tensor.matmul(
                                yp[:, 0:njb],
                                lhsT=xvt[:, jl, :],
                                rhs=attnT[jb][:, 0:njb],
                                start=(jb == 0), stop=(jb == NJB - 1),
                                skip_group_check=True)
                    yt[dm] = yt_pool.tile([P, 512], BF, tag="yt", name="yt")
                    nc.scalar.copy(yt[dm][:], yp[:])

                # outT = W_ov @ yT
                for mh in range(2):
                    wpairs = []
                    for kc in range(DK):
                        wo = wov_pool.tile([P, 1024], BF, tag="wov", name="wo")
                        nc.sync.dma_start(
                            wo[:],
                            wov_e[kc * P:(kc + 1) * P, mh * 1024:(mh + 1) * 1024])
                        wpairs.append(wo)
                    for half in range(2):
                        mq = mh * 2 + half
                        op_ = [ps_pool.tile([P, 512], F32, tag="ps512", name="op")
                               for _ in range(4)]
                        for kc in range(DK):
                            for m4 in range(4):
                                nc.tensor.matmul(
                                    op_[m4][:],
                                    lhsT=wpairs[kc][:, half * 512 + m4 * P:
                                                    half * 512 + (m4 + 1) * P],
                                    rhs=yt[kc][:],
                                    start=(kc == 0), stop=(kc == DK - 1))
                        for m4 in range(4):
                            m = mq * 4 + m4
                            ot = o_pool.tile([P, 512], F32, tag="osb", name="ot")
                            nc.vector.tensor_copy(ot[:], op_[m4][:])
                            nc.sync.dma_start(out_e[m * P:(m + 1) * P, :], ot[:])

    nc.compile()
    return nc


_NC = None
_LAST_RESULTS = None


def _get_nc():
    global _NC
    if _NC is None:
        _NC = build_graph()
    return _NC


def make_in_maps(x, W_qk, W_ov):
    x = np.asarray(x, dtype=np.float32)
    W_qk = np.asarray(W_qk, dtype=np.float32)
    W_ov = np.asarray(W_ov, dtype=np.float32)

    xk = _round_fp32r(np.ascontiguousarray(x.T))                     # [D, N]
    wqk = _round_fp32r(np.ascontiguousarray(W_qk.T))                 # [d, d']
    wov = np.ascontiguousarray(W_ov.T).astype(bfloat16)              # [d, d']
    # [DK, 4, P, 8, P] value tiles: xv[dm, jb8, r, j, c] = x[(jb8*8+j)*128+r, dm*128+c]
    xv = np.ascontiguousarray(
        x.reshape(4, 8, P, DK, P).transpose(3, 0, 2, 1, 4)).astype(bfloat16)

    keys = np.arange(512, dtype=np.int64)
    in_maps = []
    for core in range(NCORES):
        chunks = [_chunk_of(core, s) for s in range(NSLOT)]
        xq = np.concatenate([x[c * P:(c + 1) * P] for c in chunks], axis=0)
        xqT = _round_fp32r(np.ascontiguousarray(xq.T))               # [D, 512]
        mask = np.empty((len(VISITS_MASKED), P, 512), dtype=np.float32)
        for v, (g, s) in enumerate(VISITS_MASKED):
            rows = chunks[s] * P + np.arange(P, dtype=np.int64)      # query idx
            kcol = g * 512 + keys                                    # key idx
            mask[v] = np.where(kcol[None, :] <= rows[:, None], 0.0, MASK_NEG)
        in_maps.append({
            "xq": xqT, "wqk": wqk, "xk": xk, "xv": xv, "wov": wov, "mask": mask,
        })
    return in_maps


def unshard(results):
    out = np.empty((N_CTX, D), dtype=np.float32)
    for core in range(NCORES):
        r = results[core]["out"]                                     # [D, 512]
        for s in range(NSLOT):
            c = _chunk_of(core, s)
            cols = slice((3 - s) * P, (4 - s) * P)
            out[c * P:(c + 1) * P, :] = r[:, cols].T
    return out


def kernel(x, W_qk, W_ov):
    global _LAST_RESULTS
    nc = _get_nc()
    in_maps = make_in_maps(x, W_qk, W_ov)
    trace = bool(os.environ.get("KERNEL_TRACE"))
    res = run_bass_kernel_spmd(
        nc, in_maps, core_ids=list(range(NCORES)), trace=trace)
    _LAST_RESULTS = res
    return unshard(res.results)

